# revision 5
# baseline (speedup 1.0000x reference)
"""BsplineKAN fused kernel for Trainium2 (8 NeuronCores, batch-sharded), v2.

Math (per reference):
  basis = truncated in-place Cox-de Boor, degree 3, K=11 uniform knots on [0,1]
  out   = LN(einsum('bik,oik->bo', basis, cp) + x @ W.T + b) * gamma + beta

Closed form (u = 11*x, s_m = relu(u - m)):
  cubic col k (k=0..7) = s_k^3 - 4 s_{k+1}^3 + 6 s_{k+2}^3 - 4 s_{k+3}^3
                         + s_{k+4}^3   (= 6*B_k; 1/6 folded into weights)
  quad col  = s_8^2 - 3 s_9^2 + 3 s_10^2      (= 2*B_8)
  lin col   = s_9 - 2 s_10                    (= B_9)
  sign col  = sign(u - 10)                    (= 2*B_10 - 1; +1/2 via brow)
  x col     = x itself (linear layer W)

v2 speedups over the 771.9us baseline:
  * 5 of the 8 cubic columns run as fp8e4 DoubleRow matmuls (half PE cost).
    Weight error is cancelled by an e4m3 hi+lo split across the two DoubleRow
    slots (the basis column is broadcast to both slots with a stride-0 AP);
    only the ~2.4% e4m3 feature error remains (~7.5e-3 rel-err per column).
    Scales: s-tiles carry 2x (relu scale 22) so cubic cols carry 48x; fp8
    weights carry G/48 with G = 2^17; bf16 chunk weights carry G exactly
    (power-of-2, lossless). LayerNorm is scale-invariant to G.
  * basis compute in mega-ops over a contiguous [128, 12*512] layout:
    11 ACT relus + 1 ACT mega-square (into S3) + in-place GPSIMD cube +
    4-pass DVE STT difference chain with dtype-split finals.
  * LN stats via ACT accumulators (accum_out) during psum->SBUF copy/square.
  * all DMAs issue from the SP queue in an order whose waits are already
    resolved (x prefetched one i-block ahead; output DMAs deferred with the
    epilogue into the next macro) — the SP sequencer performs DMA waits
    while holding the queue, so a blocked DMA stalls every later one.
  * gamma==1/beta==0 (true here) specializes away gamma/beta ops.
"""

import functools
import numpy as np
import ml_dtypes

BATCH = 16384
INF = 1024
OUTF = 1024
NCORES = 8
BC = BATCH // NCORES        # 2048 batch rows per core
BMS = 512                   # batch-macro size
NBM = BC // BMS             # 4 macros
IB = INF // 128             # 8 i-blocks
EPS = 1e-5

FP8_COLS = (3, 4, 5, 6, 7)          # cubic cols done in fp8 DoubleRow
BF_CUBIC = tuple(k for k in range(8) if k not in FP8_COLS)
NF8 = len(FP8_COLS)
NBC = len(BF_CUBIC)
# bf16 chunk slot order: bf-cubic cols..., quad, lin, sign, x
NBF = NBC + 4
SS = 2.0                    # s-tile scale (relu scale = 22)
COL_S = 6.0 * SS ** 3       # 48: scale carried by cubic cols
G = 131072.0                # 2^17 global output scale (LN-invariant)
WS8 = G / COL_S             # 2730.67: fp8 weight scale

E4 = ml_dtypes.float8_e4m3fn
BF16NP = ml_dtypes.bfloat16


@functools.lru_cache(maxsize=2)
def _build_nc(plain_ln: bool = True):
    import concourse.mybir as mybir
    import concourse.tile as tile
    from concourse import bacc

    f32 = mybir.dt.float32
    bf16 = mybir.dt.bfloat16
    fp8 = mybir.dt.float8e4
    AF = mybir.ActivationFunctionType
    OP = mybir.AluOpType
    DR = mybir.MatmulPerfMode.DoubleRow

    nc = bacc.Bacc("TRN2", target_bir_lowering=False, debug=False)
    xT = nc.dram_tensor("xT", [INF, BC], f32, kind="ExternalInput").ap()
    cpb16 = nc.dram_tensor("cpb16", [IB * NBF * 128, OUTF], bf16,
                           kind="ExternalInput").ap()
    cpb8 = nc.dram_tensor("cpb8", [IB * NF8 * 128, 2 * OUTF], fp8,
                          kind="ExternalInput").ap()
    brow = nc.dram_tensor("brow", [2, OUTF], bf16, kind="ExternalInput").ap()
    gam = nc.dram_tensor("gam", [1, OUTF], f32, kind="ExternalInput").ap()
    bet = nc.dram_tensor("bet", [1, OUTF], f32, kind="ExternalInput").ap()
    out_d = nc.dram_tensor("out", [BC, OUTF], f32, kind="ExternalOutput").ap()

    W512 = 512

    with tile.TileContext(nc) as tc:
        from contextlib import ExitStack
        with ExitStack() as ctx:
            ep = ctx.enter_context
            consts = ep(tc.tile_pool(name="consts", bufs=1))
            xpool = ep(tc.tile_pool(name="xp", bufs=2))
            s1pool = ep(tc.tile_pool(name="s1p", bufs=2))
            s3pool = ep(tc.tile_pool(name="s3p", bufs=2))
            tpool = ep(tc.tile_pool(name="tp", bufs=2))
            qpool = ep(tc.tile_pool(name="qp", bufs=2))
            bfpool = ep(tc.tile_pool(name="bfp", bufs=2))
            f8pool = ep(tc.tile_pool(name="f8p", bufs=2))
            w16pool = ep(tc.tile_pool(name="w16p", bufs=2))
            w8pool = ep(tc.tile_pool(name="w8p", bufs=2))
            zpool = ep(tc.tile_pool(name="zp", bufs=2))
            sqpool = ep(tc.tile_pool(name="sqp", bufs=1))
            stpool = ep(tc.tile_pool(name="stp", bufs=2))
            ypool = ep(tc.tile_pool(name="yp", bufs=2))
            ppool = ep(tc.tile_pool(name="pp", bufs=8, space="PSUM"))

            brow_t = consts.tile([2, OUTF], bf16)
            nc.sync.dma_start(out=brow_t, in_=brow)
            ones_t = consts.tile([2, 128], bf16)
            nc.vector.memset(ones_t, 1.0)
            gamma_t = None
            beta_t = None
            if not plain_ln:
                gamma_t = consts.tile([128, OUTF], f32)
                nc.sync.dma_start(out=gamma_t,
                                  in_=gam.partition_broadcast(128))
                beta_t = consts.tile([128, OUTF], f32)
                nc.sync.dma_start(out=beta_t,
                                  in_=bet.partition_broadcast(128))
            # per-partition constants: col0 eps; cols 1..11: -SS*m (relu
            # biases); col 12: -10 (sign bias)
            mconst = consts.tile([128, 13], f32)
            nc.vector.memset(mconst[:, 0:1], EPS)
            for m in range(11):
                nc.vector.memset(mconst[:, m + 1:m + 2], -SS * m)
            nc.vector.memset(mconst[:, 12:13], -10.0)

            def emit_epilogue(bm, psums):
                """LayerNorm epilogue: ACT accumulators give sum(z) and
                sum(z^2) per psum bank; DVE combines; out-DMA from SP."""
                ytiles = []
                for bs_i in range(4):
                    z = zpool.tile([128, OUTF], f32, name="z")
                    zsq = sqpool.tile([128, W512], f32, name="zsq")
                    stt = stpool.tile([128, 12], f32, name="stt", tag="stt")
                    for oh in range(2):
                        nc.scalar.activation(
                            out=z[:, oh * W512:(oh + 1) * W512],
                            in_=psums[bs_i][oh], func=AF.Copy,
                            accum_out=stt[:, oh:oh + 1])
                        nc.scalar.activation(
                            out=zsq, in_=psums[bs_i][oh], func=AF.Square,
                            accum_out=stt[:, 2 + oh:3 + oh])
                    # mean = (s0+s1)/OUTF ; ex2 = (q0+q1)/OUTF
                    # var = ex2 - mean^2 ; rstd = 1/sqrt(var+eps)/G
                    nc.vector.tensor_tensor(
                        out=stt[:, 4:5], in0=stt[:, 0:1], in1=stt[:, 1:2],
                        op=OP.add)
                    nc.vector.tensor_scalar(
                        out=stt[:, 4:5], in0=stt[:, 4:5],
                        scalar1=1.0 / OUTF, scalar2=None, op0=OP.mult)
                    nc.vector.tensor_tensor(
                        out=stt[:, 5:6], in0=stt[:, 2:3], in1=stt[:, 3:4],
                        op=OP.add)
                    nc.vector.tensor_scalar(
                        out=stt[:, 5:6], in0=stt[:, 5:6],
                        scalar1=1.0 / OUTF, scalar2=None, op0=OP.mult)
                    msq = stpool.tile([128, 1], f32, name="msq", tag="stt")
                    nc.vector.tensor_tensor(
                        out=msq, in0=stt[:, 4:5], in1=stt[:, 4:5],
                        op=OP.mult)
                    nc.vector.tensor_tensor(
                        out=stt[:, 5:6], in0=stt[:, 5:6], in1=msq,
                        op=OP.subtract)
                    nc.scalar.activation(
                        out=stt[:, 6:7], in_=stt[:, 5:6], func=AF.Sqrt,
                        bias=mconst[:, 0:1], scale=1.0 / (G * G))
                    nc.vector.reciprocal(out=stt[:, 7:8], in_=stt[:, 6:7])
                    nc.vector.tensor_scalar(
                        out=stt[:, 7:8], in0=stt[:, 7:8], scalar1=1.0 / G,
                        scalar2=None, op0=OP.mult)
                    y = ypool.tile([128, OUTF], f32, name="y")
                    nc.vector.tensor_scalar(
                        out=y, in0=z, scalar1=stt[:, 4:5],
                        scalar2=stt[:, 7:8],
                        op0=OP.subtract, op1=OP.mult)
                    if not plain_ln:
                        nc.gpsimd.tensor_mul(y, y, gamma_t)
                        nc.gpsimd.tensor_add(y, y, beta_t)
                    ytiles.append((bs_i, y))
                for bs_i, y in ytiles:
                    row = bm * BMS + bs_i * 128
                    nc.sync.dma_start(out=out_d[row:row + 128, :], in_=y)

            tiles = [(bm, ib) for bm in range(NBM) for ib in range(IB)]

            def xt_dma(k):
                bm, ib = tiles[k]
                xt = xpool.tile([128, BMS], f32, name="xt")
                nc.sync.dma_start(
                    out=xt, in_=xT[ib * 128:(ib + 1) * 128,
                                   bm * BMS:(bm + 1) * BMS])
                return xt

            pending = []
            psums = None
            xts = {0: xt_dma(0)}

            def emit_tile0_strips(kk, xt, psums):
                """Tile (0,0) in four 128-wide strips so the first MMs
                start ~3x sooner (kills the 50us PE startup fill). Each
                strip q feeds exactly psum bank row bs_i=q."""
                S1 = s1pool.tile([128, 11 * BMS], f32, name="S1")
                S3 = s3pool.tile([128, 12 * BMS], f32, name="S3")
                t3 = tpool.tile([128, 8 * BMS], f32, name="tt", tag="tt")
                bslbf = bfpool.tile([128, NBF * BMS], bf16, name="bslbf")
                bsl8 = f8pool.tile([128, NF8 * BMS], fp8, name="bsl8")
                S1v = S1.rearrange("p (m b) -> p m b", m=11)
                S3v = S3.rearrange("p (m b) -> p m b", m=12)
                t3v = t3.rearrange("p (c b) -> p c b", c=8)
                bfv = bslbf.rearrange("p (c b) -> p c b", c=NBF)
                f8v = bsl8.rearrange("p (c b) -> p c b", c=NF8)
                # weight DMAs up front (no deps)
                wt8s = []
                for j0, j1 in ((0, 3), (3, NF8)):
                    nj = j1 - j0
                    wt8a = w8pool.tile([128, 3 * 2 * OUTF], fp8,
                                       name="wt8a")
                    r0 = (kk * NF8 + j0) * 128
                    nc.sync.dma_start(
                        out=wt8a[:, 0:nj * 2 * OUTF]
                        .rearrange("p (s o) -> p s o", s=nj),
                        in_=cpb8[r0:r0 + nj * 128, :]
                        .rearrange("(s p) o -> p s o", s=nj))
                    wt8s.append((j0, j1, wt8a))
                wt16s = []
                for s0, s1 in ((0, 3), (3, 5), (5, NBF)):
                    ns = s1 - s0
                    wt16a = w16pool.tile([128, 3 * OUTF], bf16,
                                         name="wt16a")
                    r0 = (kk * NBF + s0) * 128
                    nc.sync.dma_start(
                        out=wt16a[:, 0:ns * OUTF]
                        .rearrange("p (s o) -> p s o", s=ns),
                        in_=cpb16[r0:r0 + ns * 128, :]
                        .rearrange("(s p) o -> p s o", s=ns))
                    wt16s.append((s0, s1, wt16a))
                for q in range(4):
                    qs = slice(q * 128, (q + 1) * 128)
                    for m in range(11):
                        nc.scalar.activation(
                            out=S1v[:, m, qs], in_=xt[:, qs],
                            func=AF.Relu, bias=mconst[:, m + 1:m + 2],
                            scale=11.0 * SS)
                    nc.scalar.activation(out=S3v[:, 0:11, qs],
                                         in_=S1v[:, :, qs],
                                         func=AF.Square)
                    nc.gpsimd.memset(S3v[:, 11, qs], 0.0)
                    ta = qpool.tile([128, 128], f32, name="ta", tag="qa")
                    nc.gpsimd.tensor_scalar(
                        out=ta, in0=S3v[:, 9, qs], scalar1=-3.0,
                        scalar2=None, op0=OP.mult)
                    tb = qpool.tile([128, 128], f32, name="tb", tag="qa")
                    nc.gpsimd.tensor_add(tb, ta, S3v[:, 8, qs])
                    tcq = qpool.tile([128, 128], f32, name="tcq", tag="qa")
                    nc.gpsimd.tensor_scalar(
                        out=tcq, in0=S3v[:, 10, qs], scalar1=3.0,
                        scalar2=None, op0=OP.mult)
                    nc.gpsimd.tensor_mul(S3v[:, 0:11, qs],
                                         S3v[:, 0:11, qs], S1v[:, :, qs])
                    nc.vector.scalar_tensor_tensor(
                        out=t3v[:, :, qs], in0=S3v[:, 1:9, qs],
                        scalar=-4.0, in1=S3v[:, 0:8, qs],
                        op0=OP.mult, op1=OP.add)
                    nc.vector.scalar_tensor_tensor(
                        out=t3v[:, :, qs], in0=S3v[:, 2:10, qs],
                        scalar=6.0, in1=t3v[:, :, qs],
                        op0=OP.mult, op1=OP.add)
                    nc.vector.scalar_tensor_tensor(
                        out=t3v[:, :, qs], in0=S3v[:, 3:11, qs],
                        scalar=-4.0, in1=t3v[:, :, qs],
                        op0=OP.mult, op1=OP.add)
                    nc.vector.scalar_tensor_tensor(
                        out=bfv[:, 0:NBC, qs],
                        in0=S3v[:, 4:4 + NBC, qs], scalar=1.0,
                        in1=t3v[:, 0:NBC, qs], op0=OP.mult, op1=OP.add)
                    nc.vector.scalar_tensor_tensor(
                        out=f8v[:, :, qs],
                        in0=S3v[:, 4 + NBC:12, qs], scalar=1.0,
                        in1=t3v[:, NBC:8, qs], op0=OP.mult, op1=OP.add)
                    nc.gpsimd.tensor_add(bfv[:, NBC, qs], tcq, tb)
                    la = qpool.tile([128, 128], f32, name="la", tag="qa")
                    nc.gpsimd.tensor_scalar(
                        out=la, in0=S1v[:, 10, qs], scalar1=-2.0,
                        scalar2=None, op0=OP.mult)
                    nc.gpsimd.tensor_add(bfv[:, NBC + 1, qs], la,
                                         S1v[:, 9, qs])
                    nc.scalar.activation(
                        out=bfv[:, NBC + 2, qs], in_=xt[:, qs],
                        func=AF.Sign, bias=mconst[:, 12:13], scale=11.0)
                    nc.scalar.copy(bfv[:, NBC + 3, qs], xt[:, qs])
                    # strip-q MMs: all chunks for psum bank row q
                    for j0, j1, wt8a in wt8s:
                        for j in range(j0, j1):
                            rhs_pair = wt8a[:, (j - j0) * 2 * OUTF:
                                            (j - j0 + 1) * 2 * OUTF] \
                                .rearrange("p (two o) -> p two o", two=2)
                            lhsT = bsl8[:, j * BMS + q * 128:
                                        j * BMS + (q + 1) * 128] \
                                .rearrange("p (one m) -> p one m",
                                           one=1) \
                                .broadcast_to([128, 2, 128])
                            for oh in range(2):
                                nc.tensor.matmul(
                                    psums[q][oh], lhsT,
                                    rhs_pair[:, :,
                                             oh * W512:(oh + 1) * W512],
                                    start=(kk == 0 and j == 0),
                                    stop=False, perf_mode=DR)
                    for s0, s1, wt16a in wt16s:
                        for s in range(s0, s1):
                            lhsT = bslbf[:, s * BMS + q * 128:
                                         s * BMS + (q + 1) * 128]
                            for oh in range(2):
                                nc.tensor.matmul(
                                    psums[q][oh], lhsT,
                                    wt16a[:, (s - s0) * OUTF + oh * W512:
                                          (s - s0) * OUTF + oh * W512
                                          + W512],
                                    start=False, stop=False)

            for k, (bm, ib) in enumerate(tiles):
                if ib == 0:
                    psums = [[ppool.tile([128, W512], f32, name="psum",
                                         tag="psum")
                              for _ in range(2)]
                             for _ in range(4)]
                xt = xts.pop(k)
                if k + 1 < len(tiles):
                    xts[k + 1] = xt_dma(k + 1)
                if ib == 2 and pending:
                    emit_epilogue(*pending.pop(0))
                if k == 0 or k == len(tiles) - 1:
                    emit_tile0_strips(ib, xt, psums)
                    if ib == IB - 1:
                        for bs_i in range(4):
                            for oh in range(2):
                                nc.tensor.matmul(
                                    psums[bs_i][oh], ones_t,
                                    brow_t[:, oh * W512:(oh + 1) * W512],
                                    start=False, stop=True)
                        pending.append((bm, psums))
                    continue

                # S1[m] = SS * relu(11x - m), m = 0..10   (ACT)
                S1 = s1pool.tile([128, 11 * BMS], f32, name="S1")
                for m in range(11):
                    nc.scalar.activation(
                        out=S1[:, m * BMS:(m + 1) * BMS], in_=xt,
                        func=AF.Relu, bias=mconst[:, m + 1:m + 2],
                        scale=11.0 * SS)
                # S3[0:11] = S1^2 (ACT mega); quad taps read the squares,
                # then the cube multiply runs in place.
                S3 = s3pool.tile([128, 12 * BMS], f32, name="S3")
                nc.scalar.activation(out=S3[:, 0:11 * BMS], in_=S1,
                                     func=AF.Square)
                # quad col = sq8 - 3 sq9 + 3 sq10 (GPSIMD, pre-cube)
                ta = qpool.tile([128, BMS], f32, name="ta", tag="qa")
                nc.gpsimd.tensor_scalar(
                    out=ta, in0=S3[:, 9 * BMS:10 * BMS], scalar1=-3.0,
                    scalar2=None, op0=OP.mult)
                tb = qpool.tile([128, BMS], f32, name="tb", tag="qa")
                nc.gpsimd.tensor_add(tb, ta, S3[:, 8 * BMS:9 * BMS])
                # cube part 1 before the tcq tap (sq10 is only
                # overwritten by part 2) — shortens the path to t1
                nc.gpsimd.tensor_mul(S3[:, 0:9 * BMS],
                                     S3[:, 0:9 * BMS], S1[:, 0:9 * BMS])
                tcq = qpool.tile([128, BMS], f32, name="tcq", tag="qa")
                nc.gpsimd.tensor_scalar(
                    out=tcq, in0=S3[:, 10 * BMS:11 * BMS], scalar1=3.0,
                    scalar2=None, op0=OP.mult)
                nc.gpsimd.tensor_mul(S3[:, 9 * BMS:11 * BMS],
                                     S3[:, 9 * BMS:11 * BMS],
                                     S1[:, 9 * BMS:11 * BMS])
                # 4-pass difference chain (DVE STT):
                # t1[k]=S3[k]-4S3[k+1]; t2=t1+6S3[k+2]; t3=t2-4S3[k+3];
                # col[k]=t3[k]+S3[k+4]
                t1 = tpool.tile([128, 8 * BMS], f32, name="tt", tag="tt")
                nc.vector.scalar_tensor_tensor(
                    out=t1, in0=S3[:, BMS:9 * BMS], scalar=-4.0,
                    in1=S3[:, 0:8 * BMS], op0=OP.mult, op1=OP.add)
                t2 = tpool.tile([128, 8 * BMS], f32, name="tt", tag="tt")
                nc.vector.scalar_tensor_tensor(
                    out=t2, in0=S3[:, 2 * BMS:10 * BMS], scalar=6.0,
                    in1=t1, op0=OP.mult, op1=OP.add)
                t3 = tpool.tile([128, 8 * BMS], f32, name="tt", tag="tt")
                nc.vector.scalar_tensor_tensor(
                    out=t3, in0=S3[:, 3 * BMS:11 * BMS], scalar=-4.0,
                    in1=t2, op0=OP.mult, op1=OP.add)
                # final pass, dtype-split
                bslbf = bfpool.tile([128, NBF * BMS], bf16, name="bslbf")
                bsl8 = f8pool.tile([128, NF8 * BMS], fp8, name="bsl8")
                # fp8 finals first: PE consumes the fp8 chunks first,
                # so their basis must come off the DVE queue earliest
                nc.vector.scalar_tensor_tensor(
                    out=bsl8[:, 0:(NF8 - 1) * BMS],
                    in0=S3[:, (4 + NBC) * BMS:11 * BMS], scalar=1.0,
                    in1=t3[:, NBC * BMS:7 * BMS],
                    op0=OP.mult, op1=OP.add)
                nc.vector.tensor_scalar(
                    out=bsl8[:, (NF8 - 1) * BMS:NF8 * BMS],
                    in0=t3[:, 7 * BMS:8 * BMS], scalar1=1.0,
                    scalar2=None, op0=OP.mult)
                nc.vector.scalar_tensor_tensor(
                    out=bslbf[:, 0:NBC * BMS],
                    in0=S3[:, 4 * BMS:(4 + NBC) * BMS], scalar=1.0,
                    in1=t3[:, 0:NBC * BMS], op0=OP.mult, op1=OP.add)
                # finish quad col; lin col (GPSIMD)
                nc.gpsimd.tensor_add(
                    bslbf[:, NBC * BMS:(NBC + 1) * BMS], tcq, tb)
                la = qpool.tile([128, BMS], f32, name="la", tag="qa")
                nc.gpsimd.tensor_scalar(
                    out=la, in0=S1[:, 10 * BMS:11 * BMS], scalar1=-2.0,
                    scalar2=None, op0=OP.mult)
                nc.gpsimd.tensor_add(
                    bslbf[:, (NBC + 1) * BMS:(NBC + 2) * BMS], la,
                    S1[:, 9 * BMS:10 * BMS])
                # sign col + x col (ACT)
                nc.scalar.activation(
                    out=bslbf[:, (NBC + 2) * BMS:(NBC + 3) * BMS],
                    in_=xt, func=AF.Sign, bias=mconst[:, 12:13],
                    scale=11.0)
                nc.scalar.copy(
                    bslbf[:, (NBC + 3) * BMS:(NBC + 4) * BMS], xt)

                # --- matmuls ---
                first = (ib == 0)
                for j0, j1 in ((0, 3), (3, NF8)):
                    nj = j1 - j0
                    wt8a = w8pool.tile([128, 3 * 2 * OUTF], fp8,
                                       name="wt8a")
                    r0 = (ib * NF8 + j0) * 128
                    nc.sync.dma_start(
                        out=wt8a[:, 0:nj * 2 * OUTF]
                        .rearrange("p (s o) -> p s o", s=nj),
                        in_=cpb8[r0:r0 + nj * 128, :]
                        .rearrange("(s p) o -> p s o", s=nj))
                    for j in range(j0, j1):
                        rhs_pair = wt8a[:, (j - j0) * 2 * OUTF:
                                        (j - j0 + 1) * 2 * OUTF] \
                            .rearrange("p (two o) -> p two o", two=2)
                        for bs_i in range(4):
                            lhsT = bsl8[:, j * BMS + bs_i * 128:
                                        j * BMS + (bs_i + 1) * 128] \
                                .rearrange("p (one m) -> p one m", one=1) \
                                .broadcast_to([128, 2, 128])
                            for oh in range(2):
                                nc.tensor.matmul(
                                    psums[bs_i][oh], lhsT,
                                    rhs_pair[:, :,
                                             oh * W512:(oh + 1) * W512],
                                    start=(first and j == 0),
                                    stop=False, perf_mode=DR)

                for s0, s1 in ((0, 3), (3, 5), (5, NBF)):
                    ns = s1 - s0
                    wt16a = w16pool.tile([128, 3 * OUTF], bf16,
                                         name="wt16a")
                    r0 = (ib * NBF + s0) * 128
                    nc.sync.dma_start(
                        out=wt16a[:, 0:ns * OUTF]
                        .rearrange("p (s o) -> p s o", s=ns),
                        in_=cpb16[r0:r0 + ns * 128, :]
                        .rearrange("(s p) o -> p s o", s=ns))
                    for s in range(s0, s1):
                        for bs_i in range(4):
                            lhsT = bslbf[:, s * BMS + bs_i * 128:
                                         s * BMS + (bs_i + 1) * 128]
                            for oh in range(2):
                                nc.tensor.matmul(
                                    psums[bs_i][oh], lhsT,
                                    wt16a[:, (s - s0) * OUTF + oh * W512:
                                          (s - s0) * OUTF + oh * W512
                                          + W512],
                                    start=False, stop=False)

                if ib == IB - 1:
                    # bias row via ones-row matmul (closes accumulation)
                    for bs_i in range(4):
                        for oh in range(2):
                            nc.tensor.matmul(
                                psums[bs_i][oh], ones_t,
                                brow_t[:, oh * W512:(oh + 1) * W512],
                                start=False, stop=True)
                    pending.append((bm, psums))

            while pending:
                emit_epilogue(*pending.pop(0))

    nc.compile()
    return nc


def _host_prep(x, control_points, W, b):
    """Scaled weight layouts. cpb16 row ((ib*NBF+s)*128+p) holds the bf16
    weights for slot s, i = ib*128+p. cpb8 row ((ib*NF8+j)*128+p) holds
    [W0 | W1] fp8 pairs for cubic col FP8_COLS[j]."""
    cp64 = control_points.astype(np.float64)
    W64 = W.astype(np.float64)

    slot_w = []
    for kk in BF_CUBIC:
        slot_w.append(cp64[:, :, kk].T * (G / COL_S))
    slot_w.append(cp64[:, :, 8].T * (G / (2.0 * SS * SS)))   # quad
    slot_w.append(cp64[:, :, 9].T * (G / SS))                # lin
    slot_w.append(cp64[:, :, 10].T * (G / 2.0))              # sign
    slot_w.append(W64.T * G)                                 # x
    w16 = np.stack(slot_w, axis=0).reshape(NBF, IB, 128, OUTF)
    cpb16 = np.ascontiguousarray(
        w16.transpose(1, 0, 2, 3).reshape(IB * NBF * 128, OUTF)
    ).astype(BF16NP)

    w8list = []
    for kk in FP8_COLS:
        w = cp64[:, :, kk].T * WS8          # [I, O]
        assert np.abs(w).max() < 239.0, np.abs(w).max()
        W0 = w.astype(E4)
        d0 = w - W0.astype(np.float64)
        assert np.abs(d0).max() < 239.0
        W1 = d0.astype(E4)
        w8list.append(np.concatenate(
            [W0.astype(np.float64), W1.astype(np.float64)], axis=1))
    w8 = np.stack(w8list, axis=0).reshape(NF8, IB, 128, 2 * OUTF)
    cpb8 = np.ascontiguousarray(
        w8.transpose(1, 0, 2, 3).reshape(IB * NF8 * 128, 2 * OUTF)
    ).astype(E4)

    brow_f64 = (b.astype(np.float64)
                + 0.5 * cp64[:, :, 10].sum(axis=1)) * G
    brow_hi = brow_f64.astype(BF16NP)
    brow_lo = (brow_f64 - brow_hi.astype(np.float64)).astype(BF16NP)
    browa = np.ascontiguousarray(np.stack([brow_hi, brow_lo], axis=0))
    xTa = np.ascontiguousarray(x.T)  # [INF, BATCH]
    return xTa, cpb16, cpb8, browa


def kernel(x, control_points, W, b, gamma, beta):
    from concourse.bass_utils import run_bass_kernel_spmd

    xTa, cpb16, cpb8, browa = _host_prep(x, control_points, W, b)
    plain = bool(np.all(gamma == 1.0) and np.all(beta == 0.0))
    gam = np.ascontiguousarray(gamma.astype(np.float32))[None, :]
    bet = np.ascontiguousarray(beta.astype(np.float32))[None, :]

    nc = _build_nc(plain)
    in_maps = []
    for c in range(NCORES):
        in_maps.append({
            "xT": np.ascontiguousarray(xTa[:, c * BC:(c + 1) * BC]),
            "cpb16": cpb16,
            "cpb8": cpb8,
            "brow": browa,
            "gam": gam,
            "bet": bet,
        })
    res = run_bass_kernel_spmd(nc, in_maps, list(range(NCORES)))
    out = np.concatenate([res.results[c]["out"] for c in range(NCORES)],
                         axis=0)
    return out
